# revision 40
# baseline (speedup 1.0000x reference)
"""Trainium2 Bass kernel for BasicRecurrentEntityEncoder.

Math (per batch b, entity k, step t):
  enc[b,t,:]  = sum_l mask[b,t,l] * emb[prgrph[b,t,l]] * posmask[l,:]
  g           = sigmoid((h+keys)·s + (mask-1)*1e4)      (mask folded as bias:
                                                         sigmoid(-1e4) == 0)
  h_tilda     = sigmoid(h@U + keys@V + s@W)
  h           = normalize(h + g*h_tilda)

Sharding: data-parallel over batch, 8 paragraphs per core.

Phase 1 (per core): host remaps the 32768 words to compact ids (<= 32768
distinct rows) so ONE int16 dma_gather per round suffices (no dual-slice);
the compact table is bf16 (256B rows).  8 rounds of 4096 words spread over
4 SWDGE queues; each round: gather -> DVE posmask multiply (bf16) -> 32
accumulating 4-col matmuls -> ACT copy into encT.  Rounds are interleaved
with the scan: round r produces sentences t in [16r, 16r+16), and the scan
block for those t follows immediately, so gathers run under the scan.

Per-core layouts (BL=8 paragraphs, K=64, D=128 -> 512 state cols):
  feature-major: col c = b*64 + k, tiles [D=128, 512]    (for PE matmuls)
  layout-B:      chunk j = c>>7, partition p = c&127     (per-(b,k) scalars)
                 so b = 2j + (p>>6), k = p&63

Scan step (engines):
  PE : V/W matmuls issued a step early (h-independent), U matmul last as
       four consecutive 128-col matmuls (Uw stays loaded in the array),
       4 two-col gate-dot matmuls (pG[p,2j+s] = h_col . s_{2j+s}; the
       partition half picks s, so no one-hot select pass), 4+4 transposes.
  ACT: gate sigmoid, h_tilda sigmoid, 3x Square+accum_out (ss = sum hn^2),
       psum->sbuf copies of hT (half-width, so the first copy lands early).
       All funcs live in one act table set -> no table reloads.
  DVE: gate select+bias adds (mask folded as -1e4 bias), 4x gated-update
       STT, last-chunk square+reduce (so rsqrt isn't gated on ACT), 5-op
       magic-seed rsqrt + Newton, final scale (half-width).

Precision split: the matmul-side tensors (hT, encT, keysT, U/V/W, h_tilda)
are bf16 — they only feed PE contractions and the sigmoid-squashed update
— while the recurrent accumulation state hB, the gated update, and the
normalization stay f32, so rounding does not compound across the 128
steps.  Each gather round is emitted one scan step BEFORE its block
boundary so the encT/ksm writes always precede (in program order) the
next block's W-matmul that reads them — with the round emitted at the
boundary itself, that read raced the write.
"""
import numpy as np
import ml_dtypes

import concourse.bass as bass
import concourse.bacc as bacc
import concourse.tile as tile
from concourse import library_config, mybir
from concourse.bass_utils import run_bass_kernel_spmd

F32 = mybir.dt.float32
F32R = mybir.dt.float32r
BF16 = mybir.dt.bfloat16
I16 = mybir.dt.int16
I32 = mybir.dt.int32
AF = mybir.ActivationFunctionType
ALU = mybir.AluOpType

B, T, L, D, K, V = 64, 128, 32, 128, 64, 50000
NCORES = 8
BL = B // NCORES              # 8 paragraphs per core
COLS = BL * K                 # 512 state columns per core
NJ = COLS // 128              # 4 layout-B chunks
WORDS = BL * T * L            # 32768 gathered words per core
NR = 8                        # gather rounds
RW = WORDS // NR              # 4096 words per round
RC = RW // 128                # 32 chunks per round
TB = T // NR                  # 16 scan steps per round block
VSPLIT = 32768                # compact table rows (int16-addressable)
MAGIC = 0x5F3759DF
NQ = 4                        # SWDGE queues

_cache = {}


def _r(ap):
    return ap.bitcast(F32R)


def _build_nc(scan_t=T, reps=1, nq=NQ, interleave=True, sp=False):
    nc = bacc.Bacc(None, target_bir_lowering=False, num_swdge_queues=nq)

    emb_t = nc.dram_tensor("emb", [VSPLIT, D], BF16, kind="ExternalInput")
    gix_t = nc.dram_tensor("gix", [128, NR * (RW // 16)], I16,
                           kind="ExternalInput")
    mo_t = nc.dram_tensor("mo", [128, NR * RC * 4], BF16, kind="ExternalInput")
    posrep_t = nc.dram_tensor("posrep", [128, 128], BF16, kind="ExternalInput")
    keysT_t = nc.dram_tensor("keysT", [128, COLS], BF16, kind="ExternalInput")
    U_t = nc.dram_tensor("Uw", [D, D], BF16, kind="ExternalInput")
    V_t = nc.dram_tensor("Vw", [D, D], BF16, kind="ExternalInput")
    W_t = nc.dram_tensor("Ww", [D, D], BF16, kind="ExternalInput")
    mb_t = nc.dram_tensor("mbias", [128, 4 * T], F32, kind="ExternalInput")
    id_t = nc.dram_tensor("ident", [128, 128], F32R, kind="ExternalInput")
    idb_t = nc.dram_tensor("identb", [128, 128], BF16, kind="ExternalInput")
    z_t = nc.dram_tensor("zeros", [128, COLS], F32R, kind="ExternalInput")
    zb_t = nc.dram_tensor("zerosb", [128, COLS], BF16, kind="ExternalInput")
    out_t = nc.dram_tensor("h_out", [BL, K, D], F32, kind="ExternalOutput")

    RSEG = RW // 16           # 256 idx columns per round

    with tile.TileContext(nc) as tc:
        with tc.tile_pool(name="persist", bufs=1) as pp:
            posrep = pp.tile([128, 128], BF16)
            keysT = pp.tile([128, COLS], BF16)
            Uw = pp.tile([D, D], BF16)
            Vw = pp.tile([D, D], BF16)
            Ww = pp.tile([D, D], BF16)
            mbias = pp.tile([128, 4 * T], F32)
            ident = pp.tile([128, 128], F32R)
            identb = pp.tile([128, 128], BF16)
            encT = pp.tile([128, T * BL], BF16)      # [d, t*8+b]
            ksm = pp.tile([128, 4 * T], F32)         # [p, 4t+j] (mask folded)
            gix = pp.tile([128, NR * RSEG], I16)
            mo = pp.tile([128, NR * RC * 4], BF16)
            nc.sync.dma_start(out=posrep, in_=posrep_t[:, :])
            nc.sync.dma_start(out=keysT, in_=keysT_t[:, :])
            nc.sync.dma_start(out=Uw, in_=U_t[:, :])
            nc.sync.dma_start(out=Vw, in_=V_t[:, :])
            nc.sync.dma_start(out=Ww, in_=W_t[:, :])
            nc.sync.dma_start(out=mbias, in_=mb_t[:, :])
            nc.sync.dma_start(out=ident, in_=id_t[:, :])
            nc.sync.dma_start(out=identb, in_=idb_t[:, :])
            nc.sync.dma_start(out=gix, in_=gix_t[:, :])
            nc.sync.dma_start(out=mo, in_=mo_t[:, :])

            # posrep broadcast over the RC chunks of one round
            pos_bc = bass.AP(tensor=posrep.tensor, offset=posrep.offset,
                             ap=[posrep.ap[0], [0, RC], [1, 128]])

            nc.gpsimd.load_library(library_config.mlp)
            for _rep in range(reps):
              with tc.tile_pool(name="p1g", bufs=4) as p1g, \
                   tc.tile_pool(name="p1w", bufs=2) as p1w, \
                   tc.tile_pool(name="p1ps", bufs=1, space="PSUM") as p1ps, \
                   tc.tile_pool(name="ksps", bufs=1, space="PSUM") as ksps, \
                   tc.tile_pool(name="st", bufs=2) as stp, \
                   tc.tile_pool(name="sm", bufs=3) as smp, \
                   tc.tile_pool(name="scr", bufs=2) as scrp, \
                   tc.tile_pool(name="psA", bufs=2, space="PSUM") as psA, \
                   tc.tile_pool(name="psG", bufs=1, space="PSUM") as psG, \
                   tc.tile_pool(name="psB", bufs=1, space="PSUM") as psB, \
                   tc.tile_pool(name="psH", bufs=1, space="PSUM") as psH:

                def round_p1(r):
                    ga = p1g.tile([128, RC, 128], BF16, tag="ga")
                    nc.gpsimd.dma_gather(
                        ga, emb_t[:, :], gix[:, r * RSEG:(r + 1) * RSEG],
                        RW, RW, 128, elem_step=128, single_packet=sp,
                        queue_num=r % nq)
                    wa = p1w.tile([128, RC, 128], BF16, tag="wa")
                    nc.vector.tensor_tensor(out=wa, in0=ga, in1=pos_bc,
                                            op=ALU.mult)
                    penc = p1ps.tile([128, 128], F32, tag="penc")
                    for c in range(RC):
                        mcol = (r * RC + c) * 4
                        nc.tensor.matmul(
                            out=penc[:, 4 * c:4 * c + 4], lhsT=wa[:, c, :],
                            rhs=mo[:, mcol:mcol + 4],
                            start=(c == 0), stop=(c == RC - 1))
                    nc.scalar.copy(
                        out=encT[:, r * 128:(r + 1) * 128], in_=penc)
                    # ks for this round's t-block:
                    # ksm[p,4t+j] = sum_d keys[b,k,d]*enc[b,t,d] + (m-1)*1e4
                    for b in range(BL):
                        psk = ksps.tile([64, TB], F32, tag="psk")
                        encb = bass.AP(
                            tensor=encT.tensor,
                            offset=encT.offset + 128 * r + b,
                            ap=[encT.ap[0], [BL, TB]])
                        nc.tensor.matmul(out=psk,
                                         lhsT=keysT[:, b * 64:(b + 1) * 64],
                                         rhs=encb, start=True, stop=True)
                        p0 = (b & 1) * 64
                        c0 = 4 * TB * r + (b >> 1)
                        c1 = c0 + 4 * (TB - 1) + 1
                        nc.vector.tensor_tensor(
                            out=ksm[p0:p0 + 64, c0:c1:4],
                            in0=psk,
                            in1=mbias[p0:p0 + 64, c0:c1:4],
                            op=ALU.add)

                # -------------- scan state ------------------------------
                hT = stp.tile([128, COLS], BF16, tag="hT")
                hB = stp.tile([128, COLS], BF16, tag="hB")
                nc.sync.dma_start(out=hT, in_=zb_t[:, :])
                nc.sync.dma_start(out=hB, in_=zb_t[:, :])

                if interleave:
                    assert scan_t == T, "interleave requires full scan"
                else:
                    for r in range(NR):
                        round_p1(r)

                pA = None

                def issue_vw(t):
                    # h-independent matmuls for step t (keep PE warm, off
                    # the critical path)
                    p = psA.tile([128, COLS], F32, tag="pA")
                    nc.tensor.matmul(out=p, lhsT=Vw, rhs=keysT,
                                     start=True, stop=False)
                    s_bc = bass.AP(tensor=encT.tensor,
                                   offset=encT.offset + 8 * t,
                                   ap=[encT.ap[0], [1, BL], [0, K]])
                    nc.tensor.matmul(out=p, lhsT=Ww, rhs=s_bc,
                                     start=False, stop=False)
                    return p

                if interleave:
                    round_p1(0)
                for t in range(scan_t):
                    # emit round r+1 one step BEFORE its block boundary so
                    # the encT/ksm writes for block r+1 always precede (in
                    # program order) the issue_vw(t+1) that reads them at
                    # the boundary, and the gather gets a block of slack
                    if interleave and (t + 1) % TB == 0 and t + 1 < scan_t:
                        round_p1((t + 1) // TB)
                    if t == 0:
                        pA = issue_vw(0)
                    # pre-activation: += U.T @ hT, per 128-col chunk so the
                    # sigmoid/transpose/update lanes pipeline per chunk
                    # (U_0..U_3 consecutive -> Uw stays loaded in the PE)
                    for j in range(NJ):
                        nc.tensor.matmul(out=pA[:, 128 * j:128 * (j + 1)],
                                         lhsT=Uw,
                                         rhs=hT[:, 128 * j:128 * (j + 1)],
                                         start=False, stop=True,
                                         skip_group_check=True)
                    # gate dots: one 2-col matmul per chunk j gives
                    # pG[p, 2j+0/1] = dot(h col 128j+p, s_{2j}/s_{2j+1});
                    # partition half selects which of the two is kept.
                    pG = psG.tile([128, 2 * NJ], F32, tag="pG")
                    for j in range(NJ):
                        nc.tensor.matmul(
                            out=pG[:, 2 * j:2 * j + 2],
                            lhsT=hT[:, 128 * j:128 * (j + 1)],
                            rhs=encT[:, 8 * t + 2 * j:8 * t + 2 * j + 2],
                            start=True, stop=True)
                    gks = smp.tile([128, 4], F32, tag="gks")
                    nc.vector.tensor_tensor(out=gks[0:64, :],
                                            in0=pG[0:64, 0:2 * NJ:2],
                                            in1=ksm[0:64, 4 * t:4 * t + 4],
                                            op=ALU.add)
                    nc.vector.tensor_tensor(out=gks[64:128, :],
                                            in0=pG[64:128, 1:2 * NJ:2],
                                            in1=ksm[64:128, 4 * t:4 * t + 4],
                                            op=ALU.add)

                    gm = smp.tile([128, 4], F32, tag="gm")
                    nc.scalar.activation(out=gm, in_=gks, func=AF.Sigmoid)

                    htT = scrp.tile([128, COLS], BF16, tag="htT")
                    nc.scalar.activation(out=htT, in_=pA, func=AF.Sigmoid)
                    # h_tilda -> layout B
                    pB = psB.tile([128, COLS], BF16, tag="pB")
                    for j in range(NJ):
                        nc.tensor.transpose(
                            out=pB[:, 128 * j:128 * (j + 1)],
                            in_=htT[:, 128 * j:128 * (j + 1)],
                            identity=identb)
                    # hn = h + g*h_tilda (layout B); ss_j = sum_d hn^2 on ACT
                    hnB = scrp.tile([128, COLS], BF16, tag="hnB")
                    sq = scrp.tile([128, COLS], BF16, tag="sq")
                    ss = smp.tile([128, 4], F32, tag="ss")
                    for j in range(NJ):
                        nc.vector.scalar_tensor_tensor(
                            out=hnB[:, 128 * j:128 * (j + 1)],
                            in0=pB[:, 128 * j:128 * (j + 1)],
                            scalar=gm[:, j:j + 1],
                            in1=hB[:, 128 * j:128 * (j + 1)],
                            op0=ALU.mult, op1=ALU.add)
                        if j < NJ - 1:
                            # ACT square+accum pipelines behind the STTs
                            nc.scalar.activation(
                                out=sq[:, 128 * j:128 * (j + 1)],
                                in_=hnB[:, 128 * j:128 * (j + 1)],
                                func=AF.Square,
                                accum_out=ss[:, j:j + 1])
                        else:
                            # last chunk on DVE: lands right behind hnB_3 on
                            # the same queue, so rsqrt isn't gated on ACT
                            nc.vector.tensor_tensor(
                                out=sq[:, 128 * j:128 * (j + 1)],
                                in0=hnB[:, 128 * j:128 * (j + 1)],
                                in1=hnB[:, 128 * j:128 * (j + 1)],
                                op=ALU.mult)
                            nc.vector.tensor_reduce(
                                out=ss[:, j:j + 1],
                                in_=sq[:, 128 * j:128 * (j + 1)].rearrange(
                                    "p (a b) -> p a b", b=128),
                                axis=mybir.AxisListType.X, op=ALU.add)
                    # next step's V/W while the norm tail runs
                    if t + 1 < scan_t:
                        pA = issue_vw(t + 1)
                    # rsqrt via int32 magic seed + 1 Newton iteration (DVE)
                    seed = smp.tile([128, 4], I32, tag="seed")
                    nc.vector.tensor_scalar(out=seed, in0=ss.bitcast(I32),
                                            scalar1=-0.5,
                                            scalar2=float(MAGIC),
                                            op0=ALU.mult, op1=ALU.add)
                    y0 = seed.bitcast(F32)
                    t1 = smp.tile([128, 4], F32, tag="t1")
                    t2 = smp.tile([128, 4], F32, tag="t2")
                    t3 = smp.tile([128, 4], F32, tag="t3")
                    inv = smp.tile([128, 4], F32, tag="inv")
                    nc.vector.tensor_tensor(out=t1, in0=y0, in1=y0,
                                            op=ALU.mult)
                    nc.vector.tensor_tensor(out=t2, in0=t1, in1=ss,
                                            op=ALU.mult)
                    nc.vector.tensor_scalar(out=t3, in0=t2, scalar1=-0.5,
                                            scalar2=1.5, op0=ALU.mult,
                                            op1=ALU.add)
                    nc.vector.tensor_tensor(out=inv, in0=t3, in1=y0,
                                            op=ALU.mult)
                    invb = smp.tile([128, 4], BF16, tag="invb")
                    nc.vector.tensor_copy(out=invb, in_=inv)
                    # h' = hn * inv (broadcast inv over d), then -> layout T;
                    # half-width splits let the transposes/copy start on the
                    # first half while the second half is still scaling
                    hB_new = stp.tile([128, COLS], BF16, tag="hB")
                    pH = psH.tile([128, COLS], BF16, tag="pH")
                    hT_new = stp.tile([128, COLS], BF16, tag="hT")
                    for hh in range(2):
                        c0 = 256 * hh
                        inv_bc = bass.AP(tensor=invb.tensor,
                                         offset=invb.offset + 2 * hh,
                                         ap=[invb.ap[0], [1, 2], [0, 128]])
                        nc.vector.tensor_tensor(
                            out=hB_new[:, c0:c0 + 256],
                            in0=hnB[:, c0:c0 + 256],
                            in1=inv_bc, op=ALU.mult)
                        for j in (2 * hh, 2 * hh + 1):
                            nc.tensor.transpose(
                                out=pH[:, 128 * j:128 * (j + 1)],
                                in_=hB_new[:, 128 * j:128 * (j + 1)],
                                identity=identb)
                        nc.scalar.copy(out=hT_new[:, c0:c0 + 256],
                                       in_=pH[:, c0:c0 + 256])
                    hB, hT = hB_new, hT_new

                # ------ output: h = hB[(b&1)*64+k, 128*(b>>1)+:] ----------
                houtf = scrp.tile([128, COLS], F32, tag="houtf")
                nc.vector.tensor_copy(out=houtf, in_=hB)
                for b in range(BL):
                    osrc = houtf[(b & 1) * 64:(b & 1) * 64 + 64,
                                 128 * (b >> 1):128 * (b >> 1) + 128]
                    nc.sync.dma_start(out=out_t[b, :, :], in_=osrc)
    nc.compile()
    return nc


def _wrap16(seg):
    # dma_gather index layout: idx i at [i%16, i//16], replicated to 128 parts
    n = seg.shape[0]
    arr = seg.reshape(n // 16, 16).T.astype(np.int16)     # [16, n//16]
    return np.tile(arr, (8, 1))                            # [128, n//16]


def _prep_core(core, prgrph, prgrph_mask, embedding_matrix, positional_mask,
               Uw, Vw, Ww, keys):
    b0 = core * BL
    pr = prgrph[b0:b0 + BL]          # [8, T, L]
    pm = prgrph_mask[b0:b0 + BL]
    ky = keys[b0:b0 + BL]            # [8, K, D]

    vids = np.ascontiguousarray(pr.transpose(1, 0, 2)).reshape(-1)  # (t,b,l)
    uniq, inv = np.unique(vids, return_inverse=True)
    assert len(uniq) <= VSPLIT
    ctab = np.zeros((VSPLIT, D), dtype=np.float32)
    ctab[:len(uniq)] = embedding_matrix[uniq]
    idx = inv.astype(np.int16)
    gix = np.concatenate(
        [_wrap16(idx[r * RW:(r + 1) * RW]) for r in range(NR)], axis=1)

    maskf = pm.transpose(1, 0, 2).reshape(-1).astype(np.float32)
    mw = maskf.reshape(-1, 4, 32)                # [chunks, j, 32]
    nch = mw.shape[0]                            # 256 chunks

    moq = np.zeros((nch, 128, 4), dtype=np.float32)
    for jj in range(4):
        moq[:, jj * 32:(jj + 1) * 32, jj] = mw[:, jj, :]
    mo = np.ascontiguousarray(
        moq.transpose(1, 0, 2).reshape(128, nch * 4)).astype(
            ml_dtypes.bfloat16)

    posrep = np.ascontiguousarray(np.tile(positional_mask, (4, 1))).astype(
        ml_dtypes.bfloat16)
    keysT = np.ascontiguousarray(ky.transpose(2, 0, 1).reshape(D, COLS)
                                 ).astype(ml_dtypes.bfloat16)

    # layout-B: partition p, chunk j -> b = 2j + (p>>6)
    p_ar = np.arange(128)
    j_ar = np.arange(4)
    b_of = 2 * j_ar[None, :] + (p_ar[:, None] >> 6)          # [128, 4]
    msent = pm.any(axis=2).astype(np.float32)                # [8, T]
    mbias = np.ascontiguousarray(
        ((msent - 1.0) * 1e4)[b_of].transpose(0, 2, 1).reshape(128, 4 * T))
    ident = np.eye(128, dtype=np.float32)

    return {
        "emb": ctab.astype(ml_dtypes.bfloat16),
        "gix": gix, "mo": mo,
        "posrep": posrep, "keysT": keysT,
        "Uw": np.ascontiguousarray(Uw).astype(ml_dtypes.bfloat16),
        "Vw": np.ascontiguousarray(Vw).astype(ml_dtypes.bfloat16),
        "Ww": np.ascontiguousarray(Ww).astype(ml_dtypes.bfloat16),
        "mbias": mbias, "ident": ident,
        "identb": ident.astype(ml_dtypes.bfloat16),
        "zeros": np.zeros((128, COLS), dtype=np.float32),
        "zerosb": np.zeros((128, COLS), dtype=ml_dtypes.bfloat16),
    }


def kernel(prgrph, prgrph_mask, embedding_matrix, positional_mask,
           Uw, Vw, Ww, keys, _trace=False):
    prgrph = np.asarray(prgrph)
    prgrph_mask = np.asarray(prgrph_mask)
    embedding_matrix = np.asarray(embedding_matrix, dtype=np.float32)
    positional_mask = np.asarray(positional_mask, dtype=np.float32)
    Uw = np.asarray(Uw, dtype=np.float32)
    Vw = np.asarray(Vw, dtype=np.float32)
    Ww = np.asarray(Ww, dtype=np.float32)
    keys = np.asarray(keys, dtype=np.float32)

    if "nc" not in _cache:
        _cache["nc"] = _build_nc()
    nc = _cache["nc"]

    in_maps = [_prep_core(c, prgrph, prgrph_mask, embedding_matrix,
                          positional_mask, Uw, Vw, Ww, keys)
               for c in range(NCORES)]
    res = run_bass_kernel_spmd(nc, in_maps, core_ids=list(range(NCORES)),
                               trace=_trace)
    outs = [np.asarray(r["h_out"]).reshape(BL, K, D) for r in res.results]
    full = np.concatenate(outs, axis=0)
    if _trace:
        kernel.last_results = res
    return full


# revision 41
# speedup vs baseline: 1.0054x; 1.0054x over previous
"""Trainium2 Bass kernel for BasicRecurrentEntityEncoder.

Math (per batch b, entity k, step t):
  enc[b,t,:]  = sum_l mask[b,t,l] * emb[prgrph[b,t,l]] * posmask[l,:]
  g           = sigmoid((h+keys)·s + (mask-1)*1e4)      (mask folded as bias:
                                                         sigmoid(-1e4) == 0)
  h_tilda     = sigmoid(h@U + keys@V + s@W)
  h           = normalize(h + g*h_tilda)

Sharding: data-parallel over batch, 8 paragraphs per core.

Phase 1 (per core): host remaps the 32768 words to compact ids (<= 32768
distinct rows) so ONE int16 dma_gather per round suffices (no dual-slice);
the compact table is bf16 (256B rows).  8 rounds of 4096 words spread over
4 SWDGE queues; each round: gather -> DVE posmask multiply (bf16) -> 32
accumulating 4-col matmuls -> ACT copy into encT.  Rounds are interleaved
with the scan: round r produces sentences t in [16r, 16r+16), and the scan
block for those t follows immediately, so gathers run under the scan.

Per-core layouts (BL=8 paragraphs, K=64, D=128 -> 512 state cols):
  feature-major: col c = b*64 + k, tiles [D=128, 512]    (for PE matmuls)
  layout-B:      chunk j = c>>7, partition p = c&127     (per-(b,k) scalars)
                 so b = 2j + (p>>6), k = p&63

Scan step (engines):
  PE : V/W matmuls issued a step early (h-independent), U matmul last as
       four consecutive 128-col matmuls (Uw stays loaded in the array),
       4 two-col gate-dot matmuls (pG[p,2j+s] = h_col . s_{2j+s}; the
       partition half picks s, so no one-hot select pass), 4+4 transposes.
  ACT: gate sigmoid, h_tilda sigmoid, 3x Square+accum_out (ss = sum hn^2),
       psum->sbuf copies of hT (half-width, so the first copy lands early).
       All funcs live in one act table set -> no table reloads.
  DVE: gate select+bias adds (mask folded as -1e4 bias), 4x gated-update
       STT, last-chunk square+reduce (so rsqrt isn't gated on ACT), 5-op
       magic-seed rsqrt + Newton, final scale (half-width).

Precision split: the matmul-side tensors (hT, encT, keysT, U/V/W, h_tilda)
are bf16 — they only feed PE contractions and the sigmoid-squashed update
— while the recurrent accumulation state hB, the gated update, and the
normalization stay f32, so rounding does not compound across the 128
steps.  Each gather round is emitted one scan step BEFORE its block
boundary so the encT/ksm writes always precede (in program order) the
next block's W-matmul that reads them — with the round emitted at the
boundary itself, that read raced the write.
"""
import numpy as np
import ml_dtypes

import concourse.bass as bass
import concourse.bacc as bacc
import concourse.tile as tile
from concourse import library_config, mybir
from concourse.bass_utils import run_bass_kernel_spmd

F32 = mybir.dt.float32
F32R = mybir.dt.float32r
BF16 = mybir.dt.bfloat16
I16 = mybir.dt.int16
I32 = mybir.dt.int32
AF = mybir.ActivationFunctionType
ALU = mybir.AluOpType

B, T, L, D, K, V = 64, 128, 32, 128, 64, 50000
NCORES = 8
BL = B // NCORES              # 8 paragraphs per core
COLS = BL * K                 # 512 state columns per core
NJ = COLS // 128              # 4 layout-B chunks
WORDS = BL * T * L            # 32768 gathered words per core
NR = 8                        # gather rounds
RW = WORDS // NR              # 4096 words per round
RC = RW // 128                # 32 chunks per round
TB = T // NR                  # 16 scan steps per round block
VSPLIT = 32768                # compact table rows (int16-addressable)
MAGIC = 0x5F3759DF
NQ = 4                        # SWDGE queues

_cache = {}


def _r(ap):
    return ap.bitcast(F32R)


def _build_nc(scan_t=T, reps=1, nq=NQ, interleave=True, sp=False):
    nc = bacc.Bacc(None, target_bir_lowering=False, num_swdge_queues=nq)

    emb_t = nc.dram_tensor("emb", [VSPLIT, D], BF16, kind="ExternalInput")
    gix_t = nc.dram_tensor("gix", [128, NR * (RW // 16)], I16,
                           kind="ExternalInput")
    mo_t = nc.dram_tensor("mo", [128, NR * RC * 4], BF16, kind="ExternalInput")
    posrep_t = nc.dram_tensor("posrep", [128, 128], BF16, kind="ExternalInput")
    keysT_t = nc.dram_tensor("keysT", [128, COLS], BF16, kind="ExternalInput")
    U_t = nc.dram_tensor("Uw", [D, D], BF16, kind="ExternalInput")
    V_t = nc.dram_tensor("Vw", [D, D], BF16, kind="ExternalInput")
    W_t = nc.dram_tensor("Ww", [D, D], BF16, kind="ExternalInput")
    mb_t = nc.dram_tensor("mbias", [128, 4 * T], F32, kind="ExternalInput")
    id_t = nc.dram_tensor("ident", [128, 128], F32R, kind="ExternalInput")
    idb_t = nc.dram_tensor("identb", [128, 128], BF16, kind="ExternalInput")
    z_t = nc.dram_tensor("zeros", [128, COLS], F32R, kind="ExternalInput")
    zb_t = nc.dram_tensor("zerosb", [128, COLS], BF16, kind="ExternalInput")
    out_t = nc.dram_tensor("h_out", [BL, K, D], F32, kind="ExternalOutput")

    RSEG = RW // 16           # 256 idx columns per round

    with tile.TileContext(nc) as tc:
        with tc.tile_pool(name="persist", bufs=1) as pp:
            posrep = pp.tile([128, 128], BF16)
            keysT = pp.tile([128, COLS], BF16)
            Uw = pp.tile([D, D], BF16)
            Vw = pp.tile([D, D], BF16)
            Ww = pp.tile([D, D], BF16)
            mbias = pp.tile([128, 4 * T], F32)
            ident = pp.tile([128, 128], F32R)
            identb = pp.tile([128, 128], BF16)
            encT = pp.tile([128, T * BL], BF16)      # [d, t*8+b]
            ksm = pp.tile([128, 4 * T], F32)         # [p, 4t+j] (mask folded)
            gix = pp.tile([128, NR * RSEG], I16)
            mo = pp.tile([128, NR * RC * 4], BF16)
            nc.sync.dma_start(out=posrep, in_=posrep_t[:, :])
            nc.sync.dma_start(out=keysT, in_=keysT_t[:, :])
            nc.sync.dma_start(out=Uw, in_=U_t[:, :])
            nc.sync.dma_start(out=Vw, in_=V_t[:, :])
            nc.sync.dma_start(out=Ww, in_=W_t[:, :])
            nc.sync.dma_start(out=mbias, in_=mb_t[:, :])
            nc.sync.dma_start(out=ident, in_=id_t[:, :])
            nc.sync.dma_start(out=identb, in_=idb_t[:, :])
            nc.sync.dma_start(out=gix, in_=gix_t[:, :])
            nc.sync.dma_start(out=mo, in_=mo_t[:, :])

            # posrep broadcast over the RC chunks of one round
            pos_bc = bass.AP(tensor=posrep.tensor, offset=posrep.offset,
                             ap=[posrep.ap[0], [0, RC], [1, 128]])

            nc.gpsimd.load_library(library_config.mlp)
            for _rep in range(reps):
              with tc.tile_pool(name="p1g", bufs=4) as p1g, \
                   tc.tile_pool(name="p1w", bufs=2) as p1w, \
                   tc.tile_pool(name="p1ps", bufs=1, space="PSUM") as p1ps, \
                   tc.tile_pool(name="ksps", bufs=1, space="PSUM") as ksps, \
                   tc.tile_pool(name="st", bufs=2) as stp, \
                   tc.tile_pool(name="sm", bufs=3) as smp, \
                   tc.tile_pool(name="scr", bufs=2) as scrp, \
                   tc.tile_pool(name="psA", bufs=2, space="PSUM") as psA, \
                   tc.tile_pool(name="psG", bufs=1, space="PSUM") as psG, \
                   tc.tile_pool(name="psB", bufs=1, space="PSUM") as psB, \
                   tc.tile_pool(name="psH", bufs=1, space="PSUM") as psH:

                def round_p1(r):
                    ga = p1g.tile([128, RC, 128], BF16, tag="ga")
                    nc.gpsimd.dma_gather(
                        ga, emb_t[:, :], gix[:, r * RSEG:(r + 1) * RSEG],
                        RW, RW, 128, elem_step=128, single_packet=sp,
                        queue_num=r % nq)
                    wa = p1w.tile([128, RC, 128], BF16, tag="wa")
                    nc.vector.tensor_tensor(out=wa, in0=ga, in1=pos_bc,
                                            op=ALU.mult)
                    penc = p1ps.tile([128, 128], F32, tag="penc")
                    for c in range(RC):
                        mcol = (r * RC + c) * 4
                        nc.tensor.matmul(
                            out=penc[:, 4 * c:4 * c + 4], lhsT=wa[:, c, :],
                            rhs=mo[:, mcol:mcol + 4],
                            start=(c == 0), stop=(c == RC - 1))
                    nc.scalar.copy(
                        out=encT[:, r * 128:(r + 1) * 128], in_=penc)
                    # ks for this round's t-block:
                    # ksm[p,4t+j] = sum_d keys[b,k,d]*enc[b,t,d] + (m-1)*1e4
                    for b in range(BL):
                        psk = ksps.tile([64, TB], F32, tag="psk")
                        encb = bass.AP(
                            tensor=encT.tensor,
                            offset=encT.offset + 128 * r + b,
                            ap=[encT.ap[0], [BL, TB]])
                        nc.tensor.matmul(out=psk,
                                         lhsT=keysT[:, b * 64:(b + 1) * 64],
                                         rhs=encb, start=True, stop=True)
                        p0 = (b & 1) * 64
                        c0 = 4 * TB * r + (b >> 1)
                        c1 = c0 + 4 * (TB - 1) + 1
                        nc.vector.tensor_tensor(
                            out=ksm[p0:p0 + 64, c0:c1:4],
                            in0=psk,
                            in1=mbias[p0:p0 + 64, c0:c1:4],
                            op=ALU.add)

                # -------------- scan state ------------------------------
                hT = stp.tile([128, COLS], BF16, tag="hT")
                hB = stp.tile([128, COLS], F32R, tag="hB")
                nc.sync.dma_start(out=hT, in_=zb_t[:, :])
                nc.sync.dma_start(out=hB, in_=z_t[:, :])

                if interleave:
                    assert scan_t == T, "interleave requires full scan"
                else:
                    for r in range(NR):
                        round_p1(r)

                pA = None

                def issue_vw(t):
                    # h-independent matmuls for step t (keep PE warm, off
                    # the critical path)
                    p = psA.tile([128, COLS], F32, tag="pA")
                    nc.tensor.matmul(out=p, lhsT=Vw, rhs=keysT,
                                     start=True, stop=False)
                    s_bc = bass.AP(tensor=encT.tensor,
                                   offset=encT.offset + 8 * t,
                                   ap=[encT.ap[0], [1, BL], [0, K]])
                    nc.tensor.matmul(out=p, lhsT=Ww, rhs=s_bc,
                                     start=False, stop=False)
                    return p

                if interleave:
                    round_p1(0)
                for t in range(scan_t):
                    # emit round r+1 one step BEFORE its block boundary so
                    # the encT/ksm writes for block r+1 always precede (in
                    # program order) the issue_vw(t+1) that reads them at
                    # the boundary, and the gather gets a block of slack
                    if interleave and (t + 1) % TB == 0 and t + 1 < scan_t:
                        round_p1((t + 1) // TB)
                    if t == 0:
                        pA = issue_vw(0)
                    # pre-activation: += U.T @ hT, per 128-col chunk so the
                    # sigmoid/transpose/update lanes pipeline per chunk
                    # (U_0..U_3 consecutive -> Uw stays loaded in the PE)
                    for j in range(NJ):
                        nc.tensor.matmul(out=pA[:, 128 * j:128 * (j + 1)],
                                         lhsT=Uw,
                                         rhs=hT[:, 128 * j:128 * (j + 1)],
                                         start=False, stop=True,
                                         skip_group_check=True)
                    # gate dots: one 2-col matmul per chunk j gives
                    # pG[p, 2j+0/1] = dot(h col 128j+p, s_{2j}/s_{2j+1});
                    # partition half selects which of the two is kept.
                    pG = psG.tile([128, 2 * NJ], F32, tag="pG")
                    for j in range(NJ):
                        nc.tensor.matmul(
                            out=pG[:, 2 * j:2 * j + 2],
                            lhsT=hT[:, 128 * j:128 * (j + 1)],
                            rhs=encT[:, 8 * t + 2 * j:8 * t + 2 * j + 2],
                            start=True, stop=True)
                    gks = smp.tile([128, 4], F32, tag="gks")
                    nc.vector.tensor_tensor(out=gks[0:64, :],
                                            in0=pG[0:64, 0:2 * NJ:2],
                                            in1=ksm[0:64, 4 * t:4 * t + 4],
                                            op=ALU.add)
                    nc.vector.tensor_tensor(out=gks[64:128, :],
                                            in0=pG[64:128, 1:2 * NJ:2],
                                            in1=ksm[64:128, 4 * t:4 * t + 4],
                                            op=ALU.add)

                    gm = smp.tile([128, 4], F32, tag="gm")
                    nc.scalar.activation(out=gm, in_=gks, func=AF.Sigmoid)

                    htT = scrp.tile([128, COLS], BF16, tag="htT")
                    nc.scalar.activation(out=htT, in_=pA, func=AF.Sigmoid)
                    # h_tilda -> layout B
                    pB = psB.tile([128, COLS], BF16, tag="pB")
                    for j in range(NJ):
                        nc.tensor.transpose(
                            out=pB[:, 128 * j:128 * (j + 1)],
                            in_=htT[:, 128 * j:128 * (j + 1)],
                            identity=identb)
                    # hn = h + g*h_tilda (layout B); ss_j = sum_d hn^2 on ACT
                    hnB = scrp.tile([128, COLS], F32, tag="hnB")
                    sq = scrp.tile([128, COLS], F32, tag="sq")
                    ss = smp.tile([128, 4], F32, tag="ss")
                    for j in range(NJ):
                        nc.vector.scalar_tensor_tensor(
                            out=hnB[:, 128 * j:128 * (j + 1)],
                            in0=pB[:, 128 * j:128 * (j + 1)],
                            scalar=gm[:, j:j + 1],
                            in1=hB.bitcast(F32)[:, 128 * j:128 * (j + 1)],
                            op0=ALU.mult, op1=ALU.add)
                        if j < NJ - 1:
                            # ACT square+accum pipelines behind the STTs
                            nc.scalar.activation(
                                out=sq[:, 128 * j:128 * (j + 1)],
                                in_=hnB[:, 128 * j:128 * (j + 1)],
                                func=AF.Square,
                                accum_out=ss[:, j:j + 1])
                        else:
                            # last chunk on DVE: lands right behind hnB_3 on
                            # the same queue, so rsqrt isn't gated on ACT
                            nc.vector.tensor_tensor(
                                out=sq[:, 128 * j:128 * (j + 1)],
                                in0=hnB[:, 128 * j:128 * (j + 1)],
                                in1=hnB[:, 128 * j:128 * (j + 1)],
                                op=ALU.mult)
                            nc.vector.tensor_reduce(
                                out=ss[:, j:j + 1],
                                in_=sq[:, 128 * j:128 * (j + 1)].rearrange(
                                    "p (a b) -> p a b", b=128),
                                axis=mybir.AxisListType.X, op=ALU.add)
                    # next step's V/W while the norm tail runs
                    if t + 1 < scan_t:
                        pA = issue_vw(t + 1)
                    # rsqrt via int32 magic seed + 1 Newton iteration (DVE)
                    seed = smp.tile([128, 4], I32, tag="seed")
                    nc.vector.tensor_scalar(out=seed, in0=ss.bitcast(I32),
                                            scalar1=-0.5,
                                            scalar2=float(MAGIC),
                                            op0=ALU.mult, op1=ALU.add)
                    y0 = seed.bitcast(F32)
                    t1 = smp.tile([128, 4], F32, tag="t1")
                    t2 = smp.tile([128, 4], F32, tag="t2")
                    t3 = smp.tile([128, 4], F32, tag="t3")
                    inv = smp.tile([128, 4], F32, tag="inv")
                    nc.vector.tensor_tensor(out=t1, in0=y0, in1=y0,
                                            op=ALU.mult)
                    nc.vector.tensor_tensor(out=t2, in0=t1, in1=ss,
                                            op=ALU.mult)
                    nc.vector.tensor_scalar(out=t3, in0=t2, scalar1=-0.5,
                                            scalar2=1.5, op0=ALU.mult,
                                            op1=ALU.add)
                    nc.vector.tensor_tensor(out=inv, in0=t3, in1=y0,
                                            op=ALU.mult)
                    # h' = hn * inv (broadcast inv over d), then -> layout T;
                    # half-width splits let the transposes/copy start on the
                    # first half while the second half is still scaling
                    hB_new = stp.tile([128, COLS], F32R, tag="hB")
                    pH = psH.tile([128, COLS], F32, tag="pH")
                    hT_new = stp.tile([128, COLS], BF16, tag="hT")
                    for hh in range(2):
                        c0 = 256 * hh
                        inv_bc = bass.AP(tensor=inv.tensor,
                                         offset=inv.offset + 2 * hh,
                                         ap=[inv.ap[0], [1, 2], [0, 128]])
                        nc.vector.tensor_tensor(
                            out=hB_new[:, c0:c0 + 256],
                            in0=hnB[:, c0:c0 + 256],
                            in1=inv_bc, op=ALU.mult)
                        for j in (2 * hh, 2 * hh + 1):
                            nc.tensor.transpose(
                                out=_r(pH[:, 128 * j:128 * (j + 1)]),
                                in_=hB_new[:, 128 * j:128 * (j + 1)],
                                identity=ident)
                        nc.scalar.copy(out=hT_new[:, c0:c0 + 256],
                                       in_=pH[:, c0:c0 + 256])
                    hB, hT = hB_new, hT_new

                # ------ output: h = hB[(b&1)*64+k, 128*(b>>1)+:] ----------
                for b in range(BL):
                    osrc = hB.bitcast(F32)[(b & 1) * 64:(b & 1) * 64 + 64,
                                           128 * (b >> 1):128 * (b >> 1) + 128]
                    nc.sync.dma_start(out=out_t[b, :, :], in_=osrc)
    nc.compile()
    return nc


def _wrap16(seg):
    # dma_gather index layout: idx i at [i%16, i//16], replicated to 128 parts
    n = seg.shape[0]
    arr = seg.reshape(n // 16, 16).T.astype(np.int16)     # [16, n//16]
    return np.tile(arr, (8, 1))                            # [128, n//16]


def _prep_core(core, prgrph, prgrph_mask, embedding_matrix, positional_mask,
               Uw, Vw, Ww, keys):
    b0 = core * BL
    pr = prgrph[b0:b0 + BL]          # [8, T, L]
    pm = prgrph_mask[b0:b0 + BL]
    ky = keys[b0:b0 + BL]            # [8, K, D]

    vids = np.ascontiguousarray(pr.transpose(1, 0, 2)).reshape(-1)  # (t,b,l)
    uniq, inv = np.unique(vids, return_inverse=True)
    assert len(uniq) <= VSPLIT
    ctab = np.zeros((VSPLIT, D), dtype=np.float32)
    ctab[:len(uniq)] = embedding_matrix[uniq]
    idx = inv.astype(np.int16)
    gix = np.concatenate(
        [_wrap16(idx[r * RW:(r + 1) * RW]) for r in range(NR)], axis=1)

    maskf = pm.transpose(1, 0, 2).reshape(-1).astype(np.float32)
    mw = maskf.reshape(-1, 4, 32)                # [chunks, j, 32]
    nch = mw.shape[0]                            # 256 chunks

    moq = np.zeros((nch, 128, 4), dtype=np.float32)
    for jj in range(4):
        moq[:, jj * 32:(jj + 1) * 32, jj] = mw[:, jj, :]
    mo = np.ascontiguousarray(
        moq.transpose(1, 0, 2).reshape(128, nch * 4)).astype(
            ml_dtypes.bfloat16)

    posrep = np.ascontiguousarray(np.tile(positional_mask, (4, 1))).astype(
        ml_dtypes.bfloat16)
    keysT = np.ascontiguousarray(ky.transpose(2, 0, 1).reshape(D, COLS)
                                 ).astype(ml_dtypes.bfloat16)

    # layout-B: partition p, chunk j -> b = 2j + (p>>6)
    p_ar = np.arange(128)
    j_ar = np.arange(4)
    b_of = 2 * j_ar[None, :] + (p_ar[:, None] >> 6)          # [128, 4]
    msent = pm.any(axis=2).astype(np.float32)                # [8, T]
    mbias = np.ascontiguousarray(
        ((msent - 1.0) * 1e4)[b_of].transpose(0, 2, 1).reshape(128, 4 * T))
    ident = np.eye(128, dtype=np.float32)

    return {
        "emb": ctab.astype(ml_dtypes.bfloat16),
        "gix": gix, "mo": mo,
        "posrep": posrep, "keysT": keysT,
        "Uw": np.ascontiguousarray(Uw).astype(ml_dtypes.bfloat16),
        "Vw": np.ascontiguousarray(Vw).astype(ml_dtypes.bfloat16),
        "Ww": np.ascontiguousarray(Ww).astype(ml_dtypes.bfloat16),
        "mbias": mbias, "ident": ident,
        "identb": ident.astype(ml_dtypes.bfloat16),
        "zeros": np.zeros((128, COLS), dtype=np.float32),
        "zerosb": np.zeros((128, COLS), dtype=ml_dtypes.bfloat16),
    }


def kernel(prgrph, prgrph_mask, embedding_matrix, positional_mask,
           Uw, Vw, Ww, keys, _trace=False):
    prgrph = np.asarray(prgrph)
    prgrph_mask = np.asarray(prgrph_mask)
    embedding_matrix = np.asarray(embedding_matrix, dtype=np.float32)
    positional_mask = np.asarray(positional_mask, dtype=np.float32)
    Uw = np.asarray(Uw, dtype=np.float32)
    Vw = np.asarray(Vw, dtype=np.float32)
    Ww = np.asarray(Ww, dtype=np.float32)
    keys = np.asarray(keys, dtype=np.float32)

    if "nc" not in _cache:
        _cache["nc"] = _build_nc()
    nc = _cache["nc"]

    in_maps = [_prep_core(c, prgrph, prgrph_mask, embedding_matrix,
                          positional_mask, Uw, Vw, Ww, keys)
               for c in range(NCORES)]
    res = run_bass_kernel_spmd(nc, in_maps, core_ids=list(range(NCORES)),
                               trace=_trace)
    outs = [np.asarray(r["h_out"]).reshape(BL, K, D) for r in res.results]
    full = np.concatenate(outs, axis=0)
    if _trace:
        kernel.last_results = res
    return full
